# revision 2
# baseline (speedup 1.0000x reference)
"""Trainium2 Bass kernel for the STFT patch-dispatch loss — v2.

Architecture (vs baseline):
  - Host pre-folds the hop streams (vpp/vq/vm), quantizes to fp8(e4m3),
    packs sample-pairs into uint16 and ships them pre-padded; the device
    gets the [r, m] layout via one dma_start_transpose per stream (the
    2-byte XBAR transpose moves fp8 PAIRS).
  - DFT runs as fp8 DoubleRow matmuls (256-deep contraction in one
    instruction at 0.5 cyc/row) over 3 t-ranges covering t in [0, 1040).
    The t=1024 tail frame is computed on device; only the nyquist row
    (k=512) is finished on host.
  - mag^2 = re^2+im^2 via a custom DVE op (SQUARE_ADD) straight from
    PSUM for chunks 1..3; chunk 0 goes ACT Square + Pool add to balance
    engines. Sqrt on ACT.
  - Patch stage: subs split DVE/Pool, |d| fused into the DVE grouped
    reduce, k-groups summed via a small ones4 matmul.

Sharding: batch rows 2c, 2c+1 -> core c (8 cores).
"""
import numpy as np
import ml_dtypes

import concourse.bass as bass
import concourse.bacc as bacc
import concourse.mybir as mybir
from concourse import tile

dt = mybir.dt
Alu = mybir.AluOpType
Act = mybir.ActivationFunctionType
F8 = ml_dtypes.float8_e4m3

B, L = 16, 262144
NCORES = 8
RPC = B // NCORES
NFFT, HOP, PS = 1024, 256, 16
PAD = NFFT // 2
LP = L + 2 * PAD            # 263168
NBLK = LP // HOP            # 1028 blocks
T = 1025                    # real frames
TPAD = 1040                 # padded t (65 patches * 16)
MB = 1040                   # shipped blocks (mult of 16)
NPF, NPT = 33, 65
KSEL = max(1, int(NPF * NPT * 0.3))
EPS = 1e-08

TDEV = 1024                 # frames computed on device (t=1024 tail on host)
NPD = 64                    # device t-patches
import os
_HLMODE = int(os.environ.get("KHL", "3"))
HL_PRODUCTS = [("h", "h"), ("h", "l"), ("l", "h")][:_HLMODE]
ACT_SQ_CHUNKS = (0, 1)      # chunks squared on ACT (rest: DVE custom op)
POOL_ADD_CHUNKS = (0, 1, 2)  # chunks whose re2+im2 add runs on Pool
SIGS = "stg"
STREAMS = ("vph", "vpl", "vqh", "vql", "vmh", "vml")

# ---------------------------------------------------------------- custom DVE
_DVE_REGISTERED = {}


def _register_op(name, spec):
    import concourse.dve_ops as dve_ops
    from concourse.dve_spec import lower, _has_src1
    from concourse.dve_uop import DveOpSpec

    if name in _DVE_REGISTERED:
        return _DVE_REGISTERED[name]
    shas = {}
    for ver in ("v3", "v4"):
        s = DveOpSpec(name=name, opcode=0, uops=lower(spec, ver=ver),
                      rd1_en=_has_src1(spec))
        shas[ver] = s.sha(ver)
    op = dve_ops.DveOp(name, spec, subdim=False, uops_sha=shas)
    dve_ops.OPS.append(op)
    dve_ops.CUSTOM_DVE_SPECS[op.name] = op.spec
    dve_ops._SUB_OPCODE_FOR_NAME[op.name] = (
        dve_ops._CUSTOM_DVE_ROW_BASE + len(dve_ops.OPS) - 1)
    _DVE_REGISTERED[name] = op
    return op


def _register_dve_ops():
    from concourse.dve_spec import Spec, Src0, Src1, sq, maxx, scan, AluOp

    def _cumsum_ref(body_fn):
        def _r(in0, in1, c0, c1, c2):
            b = body_fn(in0.astype(np.float32), in1.astype(np.float32))
            return np.cumsum(b.reshape(b.shape[0], -1), axis=-1,
                             dtype=np.float32).reshape(b.shape)
        return _r

    sqa = _register_op("SQADD_ANT", Spec(
        body=sq(Src0) + Src1,
        reference=lambda in0, in1, c0, c1, c2: (
            in0.astype(np.float32) ** 2 + in1.astype(np.float32))))
    cad = _register_op("CUMABSDIFF_ANT", Spec(
        body=scan(AluOp.ADD, maxx(Src0 - Src1, Src1 - Src0)),
        reference=_cumsum_ref(lambda a, b: np.abs(a - b))))
    csd = _register_op("CUMSQDIFF_ANT", Spec(
        body=scan(AluOp.ADD, sq(Src0 - Src1)),
        reference=_cumsum_ref(lambda a, b: (a - b) ** 2)))
    return sqa, cad, csd


# ------------------------------------------------------------------- weights
def _chunk_w(c):
    """Base 256-sample weights for chunk c: (wc, ws) each [256, 128]."""
    r = np.arange(HOP)
    k = 4 * np.arange(128) + c
    ang = 2.0 * np.pi * np.outer(r, k) / NFFT
    return np.cos(ang), -np.sin(ang)


def _terms():
    """12 DFT product terms: (chunk, xi, weight [256,128], stream, shift)."""
    out = []
    for c in range(4):
        wc, ws = _chunk_w(c)
        if c == 0:
            out += [(c, 0, wc, "vp", 0), (c, 1, ws, "vp", 0)]
        elif c == 2:
            out += [(c, 0, wc, "vq", 0), (c, 1, ws, "vq", 0)]
        elif c == 1:
            out += [(c, 0, wc, "vm", 0), (c, 0, ws, "vm", 1),
                    (c, 1, ws, "vm", 0), (c, 1, -wc, "vm", 1)]
        else:
            out += [(c, 0, wc, "vm", 0), (c, 0, -ws, "vm", 1),
                    (c, 1, ws, "vm", 0), (c, 1, wc, "vm", 1)]
    return out


def _consts():
    """wdft_h/l [128, NT, 2, 128] fp8 (u, term, i, k) with r = 2u+i; ones4.
    hi = e4m3(w), lo = e4m3(w - hi)."""
    terms = _terms()
    nt = len(terms)
    wd = np.zeros((128, nt, 2, 128), np.float32)
    for ti, (c, xi, w, st, sh) in enumerate(terms):
        wd[:, ti, 0, :] = w[0::2, :]
        wd[:, ti, 1, :] = w[1::2, :]
    wh = wd.astype(F8)
    wl = (wd - wh.astype(np.float32)).astype(F8)
    ones4 = (np.arange(128)[:, None] // 4 ==
             np.arange(32)[None, :]).astype(np.float32)
    return {"wdfth": wh, "wdftl": wl, "ones4": ones4}


# ---------------------------------------------------------------- host packs
def _pk8(s8):
    """fp8 array [B, MB, 256] -> packed uint16-as-fp16 [B, MB, 128]."""
    pk = s8.view(np.uint8).reshape(s8.shape[0], MB, 128, 2)
    return (pk[..., 0].astype(np.uint16)
            | (pk[..., 1].astype(np.uint16) << 8)).view(np.float16)


def _pack_streams(x):
    """x [B, L] f32 -> dict of packed fp8-pair hi/lo streams [B, MB, 128]
    (fp16 container; byte 2m+i of row u holds fp8 of stream[m, r=2u+i])."""
    xp = np.pad(x, ((0, 0), (PAD, PAD)), mode="reflect")
    b = xp.reshape(x.shape[0], NBLK, HOP).astype(np.float32)
    z = np.zeros((x.shape[0], MB + 4, HOP), np.float32)
    z[:, :NBLK] = b
    vp = z[:, 0:MB] + z[:, 1:MB + 1] + z[:, 2:MB + 2] + z[:, 3:MB + 3]
    vq = z[:, 0:MB] - z[:, 1:MB + 1] + z[:, 2:MB + 2] - z[:, 3:MB + 3]
    vm = z[:, 0:MB] - z[:, 2:MB + 2]
    # zero everything past the last REAL frame's needs
    vp[:, T:] = 0.0
    vq[:, T:] = 0.0
    vm[:, T + 1:] = 0.0
    out = {}
    for nm, s in (("vp", vp), ("vq", vq), ("vm", vm)):
        sh = s.astype(F8)
        sl = (s - sh.astype(np.float32)).astype(F8)
        out[nm + "h"] = _pk8(sh)
        out[nm + "l"] = _pk8(sl)
    return out


# ----------------------------------------------------------------- device nc
def build_nc(repeat=1):
    sqa_op, cad_op, csd_op = _register_dve_ops()
    nc = bacc.Bacc("TRN2", target_bir_lowering=False, debug=False,
                   num_devices=NCORES)

    st_d = {}
    for s in SIGS:
        for nm in STREAMS:
            st_d[(s, nm)] = nc.dram_tensor(
                f"{nm}{s}", [RPC, MB, 128], dt.float16, kind="ExternalInput")
    wdft_d = {h: nc.dram_tensor(f"wdft{h}", [128, 12, 2, 128], dt.float8e4,
                                kind="ExternalInput") for h in "hl"}
    ones4_d = nc.dram_tensor("ones4", [128, 32], dt.float32,
                             kind="ExternalInput")
    osum_d = nc.dram_tensor("osum", [RPC, 3, 32, NPD], dt.float32,
                            kind="ExternalOutput")

    terms = _terms()
    # per (c, xi): list of (term_idx, stream, shift)
    cx = {(c, xi): [] for c in range(4) for xi in range(2)}
    for ti, (c, xi, w, stream, sh) in enumerate(terms):
        cx[(c, xi)].append((ti, stream, sh))

    DR = mybir.MatmulPerfMode.DoubleRow

    with tile.TileContext(nc) as tc:
        with (
            tc.tile_pool(name="const", bufs=1) as cp,
            tc.tile_pool(name="vstr", bufs=4) as vp_,
            tc.tile_pool(name="sqp", bufs=2) as sqp,
            tc.tile_pool(name="magp", bufs=2) as magp,
            tc.tile_pool(name="dp", bufs=2) as dp_,
            tc.tile_pool(name="redp", bufs=2) as redp,
            tc.tile_pool(name="outp", bufs=2) as outp,
            tc.tile_pool(name="dft_ps", bufs=3, space="PSUM") as dft_ps,
            tc.tile_pool(name="pk_ps", bufs=2, space="PSUM") as pk_ps,
        ):
            wdft = {}
            for h in "hl":
                wdft[h] = cp.tile([128, 12, 2, 128], dt.float8e4,
                                  tag=f"wdft{h}", name=f"wdft{h}")
                nc.sync.dma_start(wdft[h][:], wdft_d[h][:])
            ones4 = cp.tile([128, 32], dt.float32, tag="ones4", name="ones4")
            nc.sync.dma_start(ones4[:], ones4_d[:])

            dma_engines = (nc.sync, nc.scalar)

            def load_streams(s, b):
                """6 packed streams -> SBUF [128, MB] fp16 each."""
                tiles = {}
                for si, nm in enumerate(STREAMS):
                    t_ = vp_.tile([128, MB], dt.float16, tag=f"st_{nm}",
                                  name=f"st_{nm}")
                    dma_engines[0].dma_start_transpose(
                        t_[:], st_d[(s, nm)][b])
                    tiles[nm] = t_[:].bitcast(dt.float8e4).rearrange(
                        "p (m i) -> p i m", i=2)
                return tiles

            def signal_mags(s, b):
                v8 = load_streams(s, b)
                mag = magp.tile([128, 4, TDEV], dt.float16, tag=f"mag{s}",
                                name=f"mag{s}")

                def dft_ps_tile(c, xi):
                    ps = dft_ps.tile([128, TDEV], dt.float32, tag="dft",
                                     name="dft")
                    tl = cx[(c, xi)]
                    nmm = len(HL_PRODUCTS) * len(tl)
                    for half in range(2):
                        h0 = 512 * half
                        k = 0
                        for ti, stream, sh in tl:
                            for wh, sh8 in HL_PRODUCTS:
                                nc.tensor.matmul(
                                    ps[:, h0:h0 + 512],
                                    wdft[wh][:, ti, :, :],
                                    v8[stream + sh8][:, :,
                                                     sh + h0:sh + h0 + 512],
                                    start=(k == 0), stop=(k == nmm - 1),
                                    perf_mode=DR)
                                k += 1
                    return ps

                for c in range(4):
                    # im: PE -> ACT Square -> sq_im (fp16)
                    ps_im = dft_ps_tile(c, 1)
                    sq = sqp.tile([128, TDEV], dt.float16, tag="sq",
                                  name="sq")
                    nc.scalar.activation(sq[:], ps_im[:], Act.Square)
                    ps_re = dft_ps_tile(c, 0)
                    if c == 0:
                        # rebalance: both squares on ACT, add on DVE (2x)
                        sq2 = sqp.tile([128, TDEV], dt.float16, tag="sq2",
                                       name="sq2")
                        nc.scalar.activation(sq2[:], ps_re[:], Act.Square)
                        nc.vector.tensor_add(mag[:, c, :], sq[:], sq2[:])
                    else:
                        # re: PE -> fused DVE re^2 + sq_im -> mag2
                        nc.vector._custom_dve(sqa_op, out=mag[:, c, :],
                                              in0=ps_re[:], in1=sq[:])
                # sqrt in 2 halves (c 0,1 then 2,3)
                for h in range(2):
                    nc.scalar.activation(mag[:, 2 * h:2 * h + 2, :],
                                         mag[:, 2 * h:2 * h + 2, :], Act.Sqrt)
                return mag

            def patch(b, ms, mt, mg):
                osb = outp.tile([32, 3, NPD], dt.float32, tag="osb",
                                name="osb")
                NG = 4 * NPD
                for mi, (ta, tb, sqr) in enumerate(
                        ((ms, mg, False), (mt, mg, False), (ms, mt, True))):
                    # cumulative |a-b| (or (a-b)^2) along the whole stream
                    cum = dp_.tile([128, 4, TDEV], dt.float32, tag="cum",
                                   name=f"cum{mi}")
                    nc.vector._custom_dve(csd_op if sqr else cad_op,
                                          out=cum[:], in0=ta[:], in1=tb[:])
                    # gather every 16th running total, then shifted diff
                    ends = redp.tile([128, NG], dt.float32, tag="ends",
                                     name="ends")
                    nc.vector.tensor_copy(
                        ends[:], cum[:].rearrange("p c (a t) -> p (c a) t",
                                                  t=16)[:, :, 15])
                    red = redp.tile([128, NG], dt.float32, tag="red",
                                    name=f"red{mi}")
                    nc.vector.tensor_copy(red[:, 0:1], ends[:, 0:1])
                    nc.vector.tensor_sub(red[:, 1:NG], ends[:, 1:NG],
                                         ends[:, 0:NG - 1])
                    # fold the 4 chunks, then 4-partition groups via ones4
                    rv = red[:].rearrange("p (c a) -> p c a", c=4)
                    r2 = redp.tile([128, 2, NPD], dt.float32, tag="r2",
                                   name="r2")
                    nc.vector.tensor_add(r2[:], rv[:, 0:2, :], rv[:, 2:4, :])
                    r1 = redp.tile([128, NPD], dt.float32, tag="r1",
                                   name="r1")
                    nc.vector.tensor_add(r1[:], r2[:, 0, :], r2[:, 1, :])
                    pk = pk_ps.tile([32, NPD], dt.float32, tag="pk",
                                    name="pk")
                    nc.tensor.matmul(pk[:], ones4[:], r1[:],
                                     start=True, stop=True)
                    nc.vector.tensor_copy(osb[:, mi, :], pk[:])
                nc.sync.dma_start(osum_d[b].rearrange("m g f -> g m f"),
                                  osb[:])

            def body():
                states = []
                for b in range(RPC):
                    ms = signal_mags("s", b)
                    mt = signal_mags("t", b)
                    mg = signal_mags("g", b)
                    states.append((b, ms, mt, mg))
                for st in states:
                    patch(*st)

            if repeat == 1:
                body()
            else:
                with tc.For_i(0, repeat, 1):
                    body()

    nc.compile()
    return nc


_NC_CACHE = {}


def _get_nc():
    if "nc" not in _NC_CACHE:
        _NC_CACHE["nc"] = build_nc()
    return _NC_CACHE["nc"]


def _make_in_maps(xs, xt, xg):
    consts = _consts()
    packs = {s: _pack_streams(x) for s, x in (("s", xs), ("t", xt),
                                              ("g", xg))}
    in_maps = []
    for c in range(NCORES):
        m = {}
        for s in SIGS:
            for nm in STREAMS:
                m[f"{nm}{s}"] = packs[s][nm][RPC * c:RPC * (c + 1)]
        m.update(consts)
        in_maps.append(m)
    return in_maps


def _run_on_cores(nc, in_maps):
    """Execute via cached PJRT callable (axon) with jit reuse."""
    from concourse.bass_utils import axon_active

    if not axon_active():
        from concourse.bass_utils import run_bass_kernel_spmd
        return run_bass_kernel_spmd(nc, in_maps,
                                    core_ids=list(range(NCORES))).results

    import jax
    from jax.sharding import Mesh, PartitionSpec
    from jax.experimental.shard_map import shard_map
    from concourse import bass2jax

    key = id(nc)
    if key not in _NC_CACHE.setdefault("jit", {}):
        bass2jax.install_neuronx_cc_hook()
        part_name = (nc.partition_id_tensor.name
                     if nc.partition_id_tensor else None)
        in_names, out_names, out_avals, zero_outs = [], [], [], []
        for alloc in nc.m.functions[0].allocations:
            if not isinstance(alloc, mybir.MemoryLocationSet):
                continue
            name = alloc.memorylocations[0].name
            if alloc.kind == "ExternalInput":
                if name != part_name:
                    in_names.append(name)
            elif alloc.kind == "ExternalOutput":
                shape = tuple(alloc.tensor_shape)
                dtype = mybir.dt.np(alloc.dtype)
                out_names.append(name)
                out_avals.append(jax.core.ShapedArray(shape, dtype))
                zero_outs.append(np.zeros(shape, dtype))
        n_params = len(in_names)
        all_names = in_names + out_names
        if part_name is not None:
            all_names = all_names + [part_name]

        def _body(*args):
            operands = list(args)
            if part_name is not None:
                operands.append(bass2jax.partition_id_tensor())
            outs = bass2jax._bass_exec_p.bind(
                *operands, out_avals=tuple(out_avals),
                in_names=tuple(all_names), out_names=tuple(out_names),
                lowering_input_output_aliases=(),
                sim_require_finite=True, sim_require_nnan=True, nc=nc)
            return tuple(outs)

        devices = jax.devices()[:NCORES]
        mesh = Mesh(np.asarray(devices), ("core",))
        n_outs = len(out_names)
        sharded = jax.jit(
            shard_map(_body, mesh=mesh,
                      in_specs=(PartitionSpec("core"),) * (n_params + n_outs),
                      out_specs=(PartitionSpec("core"),) * n_outs,
                      check_rep=False),
            donate_argnums=tuple(range(n_params, n_params + n_outs)),
            keep_unused=True)
        _NC_CACHE["jit"][key] = (sharded, in_names, out_names, out_avals,
                                 zero_outs)

    sharded, in_names, out_names, out_avals, zero_outs = _NC_CACHE["jit"][key]
    concat_in = [np.concatenate([m[n] for m in in_maps], axis=0)
                 for n in in_names]
    concat_zeros = [np.zeros((NCORES * z.shape[0], *z.shape[1:]), z.dtype)
                    for z in zero_outs]
    out_arrs = sharded(*concat_in, *concat_zeros)
    return [
        {n: np.asarray(out_arrs[i]).reshape(NCORES, *out_avals[i].shape)[c]
         for i, n in enumerate(out_names)}
        for c in range(NCORES)
    ]


# ------------------------------------------------------------- host finishing
def _host_edges(x):
    """Edge spectra from raw waveform x [B, L]: (tail |X[:, 1024]| [B, 513],
    nyq |X[512, :]| [B, T])."""
    xp = np.pad(x.astype(np.float64), ((0, 0), (PAD, PAD)), mode="reflect")
    tail = np.abs(np.fft.rfft(xp[:, TDEV * HOP:TDEV * HOP + NFFT], axis=-1))
    alt = xp * np.where(np.arange(LP) % 2 == 0, 1.0, -1.0)[None, :]
    S = np.cumsum(alt, axis=-1)
    idx_hi = np.arange(T) * HOP + NFFT - 1
    nyq = S[:, idx_hi].copy()
    nz = np.arange(1, T) * HOP - 1
    nyq[:, 1:] -= S[:, nz]
    return np.maximum(tail, EPS), np.maximum(np.abs(nyq), EPS)


def _host_finish(osum, xs, xt, xg):
    """osum [B, 3, 32, NPD] from device + host tail column and nyquist row."""
    inv = np.float32(1.0 / (PS * PS))
    edges = {s: _host_edges(x) for s, x in (("s", xs), ("t", xt), ("g", xg))}

    sums = np.zeros((B, 3, NPF, NPT), np.float64)
    sums[:, :, :32, :NPD] = osum
    for mi, (a, b_, sqr) in enumerate(
            (("s", "g", False), ("t", "g", False), ("s", "t", True))):
        ta_, na = edges[a]
        tb_, nb = edges[b_]
        da = ta_[:, :512] - tb_[:, :512]          # [B, 512] tail col
        va = da ** 2 if sqr else np.abs(da)
        sums[:, mi, :32, NPD] = va.reshape(B, 32, 16).sum(axis=2)
        dn = na - nb                              # [B, T] nyq row
        vn = dn ** 2 if sqr else np.abs(dn)
        vz = np.zeros((B, NPT * PS))
        vz[:, :T] = vn
        sums[:, mi, 32, :] = vz.reshape(B, NPT, PS).sum(axis=2)

    s2 = sums.reshape(B, 3, NPF * NPT).astype(np.float32)
    err_s = s2[:, 0] * inv
    err_t = s2[:, 1] * inv
    pl = s2[:, 2] * inv
    kgs = err_s - err_t
    order = np.argsort(-kgs, axis=1, kind="stable")[:, :KSEL]
    mask = np.zeros_like(kgs)
    np.put_along_axis(mask, order, 1.0, axis=1)
    selected = (pl * mask).sum(axis=1, dtype=np.float32)
    count = np.maximum(mask.sum(axis=1, dtype=np.float32), 1.0)
    loss = np.float32(np.mean(selected / count, dtype=np.float32))
    sel_ratio = np.float32(mask.mean(dtype=np.float32))
    kgs_mean = np.float32(kgs.mean(dtype=np.float32))
    kgs_pos_ratio = np.float32((kgs > 0).mean(dtype=np.float32))
    return loss, sel_ratio, kgs_mean, kgs_pos_ratio


def kernel(student_waveform, teacher_waveform, target_waveform,
           n_fft=1024, hop_length=256, patch_size=16):
    xs = np.ascontiguousarray(student_waveform, dtype=np.float32)
    xt = np.ascontiguousarray(teacher_waveform, dtype=np.float32)
    xg = np.ascontiguousarray(target_waveform, dtype=np.float32)

    nc = _get_nc()
    in_maps = _make_in_maps(xs, xt, xg)
    results = _run_on_cores(nc, in_maps)
    osum = np.concatenate([r["osum"] for r in results], axis=0)  # [B,3,32,NPT]
    return _host_finish(osum, xs, xt, xg)
